# revision 6
# baseline (speedup 1.0000x reference)
"""Trainium2 Bass kernel for nn_CSPLayer (GNN message passing layer).

Strategy (8-core SPMD, single program, per-core data):
 - Host sorts edges by src (= edge_index[0], the scatter key) and shards
   nodes into 8 contiguous 6272-node ranges; each core owns all edges whose
   src falls in its range, so the scatter-mean needs no cross-core reduce.
 - Edge layer-1 input decomposes: z1 = P1[src] + P2[dst] + [lat,fd,1]@W1cd10
   where P1 = NF@W1a, P2 = NF@W1b are node-space projections (computed on
   device), and the lattice/frac_diff/bias contribution is a rank-10 matmul
   (lat6 = lattices[edge2graph] expanded on host - pure input relayout).
 - P1[src]: src is sorted, so each 128-edge tile hits one aligned 128-node
   window -> expand P1 window rows with a one-hot matmul (no DMA gather).
 - P2[dst]: random access -> indirect DMA row gather (the one unavoidable
   per-edge random access).
 - Scatter-mean: one-hot matmul (transposed: PSUM holds [feat, node-window])
   accumulated over a window's tiles; invcnt folded in per edge.
 - Node MLP + residual computed feature-major on device, output re-transposed.

Program structure is identical on all cores: every window is padded to a
fixed number T of 128-edge tiles with sentinel edges (srcloc=-1 -> one-hot
row is zero -> contributes nothing).
"""

import numpy as np

import concourse.bass as bass
import concourse.mybir as mybir
import concourse.tile as tile
from concourse import bacc
from concourse.bass_utils import run_bass_kernel_spmd

N_CORES = 8
H = 128
P = 128
WPC = 49            # windows per core (49*128 = 6272 nodes per core)
RPC = WPC * P       # nodes per core (padded; 8*6272 = 50176 >= 50000)
F32 = mybir.dt.float32
I32 = mybir.dt.int32


def _build_program(T, n_nodes):
    """Build the SPMD Bass program. T = tiles per window (fixed across cores)."""
    nc = bacc.Bacc()
    EPC = WPC * T * P          # padded edges per core
    NPAD = WPC * P * N_CORES   # padded node count for P2 table

    # ---- DRAM tensors (per-core inputs) ----
    nfT = nc.dram_tensor("nfT", [P, NPAD], F32, kind="ExternalInput")          # full NF^T (zero-padded cols)
    nfT_loc = nc.dram_tensor("nfT_loc", [P, RPC], F32, kind="ExternalInput")   # this core's NF^T slice
    w1a = nc.dram_tensor("w1a", [P, H], F32, kind="ExternalInput")
    w1b = nc.dram_tensor("w1b", [P, H], F32, kind="ExternalInput")
    w1cd = nc.dram_tensor("w1cd", [10, H], F32, kind="ExternalInput")          # [W1c; W1d; b1]
    w2 = nc.dram_tensor("w2", [H, H], F32, kind="ExternalInput")
    b2row = nc.dram_tensor("b2row", [1, H], F32, kind="ExternalInput")
    nw1 = nc.dram_tensor("nw1", [2 * H, H], F32, kind="ExternalInput")
    nb1c = nc.dram_tensor("nb1c", [H, 1], F32, kind="ExternalInput")
    nw2 = nc.dram_tensor("nw2", [H, H], F32, kind="ExternalInput")
    nb2c = nc.dram_tensor("nb2c", [H, 1], F32, kind="ExternalInput")
    ident = nc.dram_tensor("ident", [P, P], F32, kind="ExternalInput")
    iotaF = nc.dram_tensor("iotaF", [P, P], F32, kind="ExternalInput")         # iotaF[p, j] = j
    ones1 = nc.dram_tensor("ones1", [1, P], F32, kind="ExternalInput")
    srccol = nc.dram_tensor("srccol", [P, WPC * T], F32, kind="ExternalInput")  # window-local src (or -1)
    invc = nc.dram_tensor("invc", [P, WPC * T], F32, kind="ExternalInput")      # 1/max(cnt,1) per edge (0 pad)
    dsti = nc.dram_tensor("dsti", [P, WPC * T], I32, kind="ExternalInput")      # dst node idx per edge
    lat10 = nc.dram_tensor("lat10", [10, EPC], F32, kind="ExternalInput")       # [lat6; fd3; 1] per edge, 0 pad

    p2 = nc.dram_tensor("p2", [NPAD, H], F32)                                   # internal scratch
    out = nc.dram_tensor("out", [RPC, H], F32, kind="ExternalOutput")

    with tile.TileContext(nc) as tc:
        with (
            tc.tile_pool(name="const", bufs=1) as cpool,
            tc.tile_pool(name="persist", bufs=1) as ppool,
            tc.tile_pool(name="work", bufs=3) as wpool,
            tc.tile_pool(name="gath", bufs=12) as gpool,
            tc.tile_pool(name="lat", bufs=3) as lpool,
            tc.tile_pool(name="ps", bufs=1, space="PSUM") as pspool,
            tc.tile_pool(name="psagg", bufs=2, space="PSUM") as paggpool,
            tc.tile_pool(name="mlp", bufs=2) as mpool,
        ):
            # ---- load constants ----
            ic = cpool.tile([P, P], F32, tag="ident")
            nc.sync.dma_start(out=ic[:], in_=ident[:])
            iof = cpool.tile([P, P], F32, tag="iotaF")
            nc.sync.dma_start(out=iof[:], in_=iotaF[:])
            on1 = cpool.tile([1, P], F32, tag="ones1")
            nc.sync.dma_start(out=on1[:], in_=ones1[:])
            w1a_s = cpool.tile([P, H], F32, tag="w1a")
            nc.sync.dma_start(out=w1a_s[:], in_=w1a[:])
            w1b_s = cpool.tile([P, H], F32, tag="w1b")
            nc.sync.dma_start(out=w1b_s[:], in_=w1b[:])
            w1cd_s = cpool.tile([10, H], F32, tag="w1cd")
            nc.sync.dma_start(out=w1cd_s[:], in_=w1cd[:])
            w2_s = cpool.tile([H, H], F32, tag="w2")
            nc.sync.dma_start(out=w2_s[:], in_=w2[:])
            b2_s = cpool.tile([1, H], F32, tag="b2row")
            nc.sync.dma_start(out=b2_s[:], in_=b2row[:])
            nw1_s = cpool.tile([H, 2 * H], F32, tag="nw1")
            nc.sync.dma_start(out=nw1_s[:, 0:H], in_=nw1[0:H])
            nc.sync.dma_start(out=nw1_s[:, H:2 * H], in_=nw1[H:2 * H])
            nb1_s = cpool.tile([H, 1], F32, tag="nb1c")
            nc.sync.dma_start(out=nb1_s[:], in_=nb1c[:])
            nw2_s = cpool.tile([H, H], F32, tag="nw2")
            nc.sync.dma_start(out=nw2_s[:], in_=nw2[:])
            nb2_s = cpool.tile([H, 1], F32, tag="nb2c")
            nc.sync.dma_start(out=nb2_s[:], in_=nb2c[:])
            src_s = cpool.tile([P, WPC * T], F32, tag="srccol")
            nc.sync.dma_start(out=src_s[:], in_=srccol[:])
            inv_s = cpool.tile([P, WPC * T], F32, tag="invc")
            nc.sync.dma_start(out=inv_s[:], in_=invc[:])
            dst_s = cpool.tile([P, WPC * T], I32, tag="dsti")
            nc.sync.dma_start(out=dst_s[:], in_=dsti[:])

            # ---- persistent SBUF ----
            nfl = ppool.tile([P, RPC], F32, tag="nfl")       # local NF^T  [f, n]
            nc.sync.dma_start(out=nfl[:], in_=nfT_loc[:])
            p1 = ppool.tile([P, RPC], F32, tag="p1")         # P1 windows, node-major [n%128, w*128+f]
            aggT = ppool.tile([P, RPC], F32, tag="aggT")     # agg, feature-major [f, n]

            # ---- prologue: P2 = NF @ W1b -> DRAM (node-major rows) ----
            NW_ALL = NPAD // P
            GB = 4  # windows per store batch
            for g in range(NW_ALL // GB):
                pt = wpool.tile([P, GB * P], F32, tag="p2blk")
                nfb = wpool.tile([P, GB * P], F32, tag="nfb")
                nc.sync.dma_start(out=nfb[:], in_=nfT[:, g * GB * P:(g + 1) * GB * P])
                for j in range(GB):
                    ps = pspool.tile([P, P], F32, tag="psA")
                    nc.tensor.matmul(ps[:], lhsT=nfb[:, j * P:(j + 1) * P], rhs=w1b_s[:],
                                     start=True, stop=True)
                    nc.scalar.copy(out=pt[:, j * P:(j + 1) * P], in_=ps[:])
                nc.sync.dma_start(out=p2.ap().rearrange("(b n) f -> n b f", n=P)[:, g * GB:(g + 1) * GB, :],
                                  in_=pt[:])
            # ---- P1 windows for this core's range ----
            for w in range(WPC):
                ps = pspool.tile([P, P], F32, tag="psA")
                nc.tensor.matmul(ps[:], lhsT=nfl[:, w * P:(w + 1) * P], rhs=w1a_s[:],
                                 start=True, stop=True)
                nc.vector.tensor_copy(out=p1[:, w * P:(w + 1) * P], in_=ps[:])

            # ---- edge phase ----
            for w in range(WPC):
                lt = lpool.tile([10, T * P], F32, tag="lat")
                nc.sync.dma_start(out=lt[:], in_=lat10[:, w * T * P:(w + 1) * T * P])
                aggp = paggpool.tile([P, P], F32, tag="aggps")
                for t in range(T):
                    g = w * T + t
                    # one-hot [e, n]: (src_local == iota)
                    oh = wpool.tile([P, P], F32, tag="oh")
                    nc.vector.tensor_tensor(out=oh[:], in0=src_s[:, g:g + 1].to_broadcast([P, P]),
                                            in1=iof[:], op=mybir.AluOpType.is_equal)
                    # one-hot^T via PE transpose
                    ohTp = pspool.tile([P, P], F32, tag="psA")
                    nc.tensor.matmul(ohTp[:], lhsT=oh[:], rhs=ic[:], start=True, stop=True)
                    ohT = wpool.tile([P, P], F32, tag="ohT")
                    nc.scalar.copy(out=ohT[:], in_=ohTp[:])
                    # gather P2 rows for dst
                    gp2 = gpool.tile([P, H], F32, tag="gp2")
                    nc.gpsimd.indirect_dma_start(
                        out=gp2[:], out_offset=None, in_=p2[:],
                        in_offset=bass.IndirectOffsetOnAxis(ap=dst_s[:, g:g + 1], axis=0))
                    # z1 = P1-expand + lat10@W1cd + P2
                    z1p = pspool.tile([P, H], F32, tag="psB")
                    nc.tensor.matmul(z1p[:], lhsT=ohT[:], rhs=p1[:, w * P:(w + 1) * P],
                                     start=True, stop=False)
                    nc.tensor.matmul(z1p[:], lhsT=lt[:, t * P:(t + 1) * P], rhs=w1cd_s[:],
                                     start=False, stop=False)
                    nc.tensor.matmul(z1p[:], lhsT=ic[:], rhs=gp2[:], start=False, stop=True)
                    ea = wpool.tile([P, H], F32, tag="ea")
                    nc.scalar.activation(ea[:], z1p[:], mybir.ActivationFunctionType.Silu)
                    # e^T, then z2 = e @ W2 + b2 (edge-major out)
                    eTp = pspool.tile([P, P], F32, tag="psC")
                    nc.tensor.matmul(eTp[:], lhsT=ea[:], rhs=ic[:], start=True, stop=True)
                    eT = wpool.tile([P, P], F32, tag="eT")
                    nc.vector.tensor_copy(out=eT[:], in_=eTp[:])
                    z2p = pspool.tile([P, H], F32, tag="psD")
                    nc.tensor.matmul(z2p[:], lhsT=eT[:], rhs=w2_s[:], start=True, stop=False)
                    nc.tensor.matmul(z2p[:], lhsT=on1[:], rhs=b2_s[:], start=False, stop=True)
                    ef = wpool.tile([P, H], F32, tag="ef")
                    nc.scalar.activation(ef[:], z2p[:], mybir.ActivationFunctionType.Silu)
                    efs = wpool.tile([P, H], F32, tag="efs")
                    nc.vector.tensor_scalar_mul(efs[:], ef[:], inv_s[:, g:g + 1])
                    # scatter (transposed): aggp[f, n] += ef^T @ onehot
                    nc.tensor.matmul(aggp[:], lhsT=efs[:], rhs=oh[:],
                                     start=(t == 0), stop=(t == T - 1))
                nc.scalar.copy(out=aggT[:, w * P:(w + 1) * P], in_=aggp[:])

            # ---- node MLP (feature-major), residual, transpose out ----
            NG = 4  # windows per group
            for g in range(WPC // NG + (1 if WPC % NG else 0)):
                w0 = g * NG
                nw = min(NG, WPC - w0)
                L = nw * P
                sl = slice(w0 * P, w0 * P + L)
                h1p = pspool.tile([P, NG * P], F32, tag="psD")
                nc.tensor.matmul(h1p[:, :L], lhsT=nw1_s[:, 0:H], rhs=nfl[:, sl],
                                 start=True, stop=False)
                nc.tensor.matmul(h1p[:, :L], lhsT=nw1_s[:, H:2 * H], rhs=aggT[:, sl],
                                 start=False, stop=True)
                h1 = mpool.tile([P, NG * P], F32, tag="h1")
                nc.scalar.activation(h1[:, :L], h1p[:, :L],
                                     mybir.ActivationFunctionType.Silu, bias=nb1_s[:])
                h2p = pspool.tile([P, NG * P], F32, tag="psD")
                nc.tensor.matmul(h2p[:, :L], lhsT=nw2_s[:], rhs=h1[:, :L],
                                 start=True, stop=True)
                h2 = mpool.tile([P, NG * P], F32, tag="h2")
                nc.scalar.activation(h2[:, :L], h2p[:, :L],
                                     mybir.ActivationFunctionType.Silu, bias=nb2_s[:])
                oT = mpool.tile([P, NG * P], F32, tag="oT")
                nc.vector.tensor_add(out=oT[:, :L], in0=h2[:, :L], in1=nfl[:, sl])
                ob = mpool.tile([P, NG * P], F32, tag="ob")
                for j in range(nw):
                    op_ = pspool.tile([P, P], F32, tag="psA")
                    nc.tensor.matmul(op_[:], lhsT=oT[:, j * P:(j + 1) * P], rhs=ic[:],
                                     start=True, stop=True)
                    nc.vector.tensor_copy(out=ob[:, j * P:(j + 1) * P], in_=op_[:])
                nc.sync.dma_start(
                    out=out.ap().rearrange("(b n) f -> n b f", n=P)[:, w0:w0 + nw, :],
                    in_=ob[:, :L])

    nc.compile()
    return nc


def _prep_core(k, src, dst, lat10_all, invc_e, T):
    """Build core k's padded data arrays from globally sorted edge data."""
    r0, r1 = k * RPC, (k + 1) * RPC
    e0, e1 = np.searchsorted(src, [r0, r1])
    s, d = src[e0:e1], dst[e0:e1]
    l10 = lat10_all[:, e0:e1]
    ic = invc_e[e0:e1]
    EPC = WPC * T * P
    srcloc = np.full(EPC, -1.0, np.float32)
    dsti = np.zeros(EPC, np.int32)
    invc = np.zeros(EPC, np.float32)
    l10p = np.zeros((10, EPC), np.float32)
    # split this core's edges by aligned 128-node window, pad each to T*128
    wid = (s - r0) // P
    bounds = np.searchsorted(wid, np.arange(WPC + 1))
    for w in range(WPC):
        a, b = bounds[w], bounds[w + 1]
        n = b - a
        if n > T * P:
            raise RuntimeError(f"window overflow: {n} > {T * P}")
        o = w * T * P
        srcloc[o:o + n] = (s[a:b] - r0 - w * P).astype(np.float32)
        dsti[o:o + n] = d[a:b]
        invc[o:o + n] = ic[a:b]
        l10p[:, o:o + n] = l10[:, a:b]
    # [128, ntiles] layouts: column t holds edges t*128..t*128+127
    nt = WPC * T
    srccol = srcloc.reshape(nt, P).T.copy()
    dcol = dsti.reshape(nt, P).T.copy()
    iccol = invc.reshape(nt, P).T.copy()
    return srccol, dcol, iccol, l10p


def kernel(**inputs):
    inp = {k: np.asarray(v) for k, v in inputs.items()}
    nf = inp["node_features"].astype(np.float32)
    lattices = inp["lattices"].astype(np.float32)
    fd = inp["frac_diff"].astype(np.float32)
    ei = inp["edge_index"].astype(np.int64)
    e2g = inp["edge2graph"].astype(np.int64)
    e_w1, e_b1 = inp["e_w1"].astype(np.float32), inp["e_b1"].astype(np.float32)
    e_w2, e_b2 = inp["e_w2"].astype(np.float32), inp["e_b2"].astype(np.float32)
    n_w1, n_b1 = inp["n_w1"].astype(np.float32), inp["n_b1"].astype(np.float32)
    n_w2, n_b2 = inp["n_w2"].astype(np.float32), inp["n_b2"].astype(np.float32)

    N, Hf = nf.shape
    E = ei.shape[1]
    assert Hf == H and N <= N_CORES * RPC

    # ---- host-side sharding prep (sort by src; pure index/layout work) ----
    perm = np.argsort(ei[0], kind="stable")
    src = ei[0][perm].astype(np.int64)
    dst = ei[1][perm].astype(np.int32)
    e2gs = e2g[perm]
    fds = fd[perm]
    lat10_all = np.concatenate(
        [lattices[e2gs].T.astype(np.float32),
         fds.T.astype(np.float32),
         np.ones((1, E), np.float32)], axis=0)            # [10, E]
    cnt = np.bincount(src, minlength=N).astype(np.float32)
    invc_e = (1.0 / np.maximum(cnt, 1.0))[src].astype(np.float32)

    # fixed tiles-per-window across all cores
    r_all = src // P
    wcnt = np.bincount(r_all, minlength=N_CORES * WPC)
    T = max(18, int(np.ceil(wcnt.max() / P)))

    nc = _build_program(T, N)

    NPAD = N_CORES * RPC
    nfT = np.zeros((H, NPAD), np.float32)
    nfT[:, :N] = nf.T
    w1cd = np.concatenate([e_w1[2 * H:], e_b1[None, :]], axis=0)  # [10,128]
    iotaF = np.tile(np.arange(P, dtype=np.float32)[None, :], (P, 1))
    ident = np.eye(P, dtype=np.float32)

    common = dict(
        nfT=nfT, w1a=e_w1[0:H].copy(), w1b=e_w1[H:2 * H].copy(), w1cd=w1cd,
        w2=e_w2, b2row=e_b2[None, :].copy(), nw1=n_w1, nb1c=n_b1[:, None].copy(),
        nw2=n_w2, nb2c=n_b2[:, None].copy(), ident=ident, iotaF=iotaF,
        ones1=np.ones((1, P), np.float32),
    )
    in_maps = []
    for k in range(N_CORES):
        srccol, dcol, iccol, l10p = _prep_core(k, src, dst, lat10_all, invc_e, T)
        in_maps.append(dict(
            common,
            nfT_loc=np.ascontiguousarray(nfT[:, k * RPC:(k + 1) * RPC]),
            srccol=srccol, invc=iccol, dsti=dcol, lat10=l10p,
        ))

    r = run_bass_kernel_spmd(nc, in_maps, core_ids=list(range(N_CORES)),
                             trace=bool(int(__import__("os").environ.get("K_TRACE", "0"))))
    out = np.concatenate([r.results[k]["out"] for k in range(N_CORES)], axis=0)[:N]
    kernel.last_exec_ns = r.exec_time_ns
    kernel.last_mean_ns = r.mean_exec_time_ns
    return out.astype(inputs["node_features"].dtype if hasattr(inputs["node_features"], "dtype") else np.float32)
